# revision 19
# baseline (speedup 1.0000x reference)
"""Trainium2 Bass kernel for nn_DropGlobalScaledDotProductAttention.

Computation (reference semantics):
  a = d1 @ W1[:256]; c = d0 @ W1[256:]
  h[b,i,j,:] = relu(a[b,i,:] + c[b,j,:] + b1)          # [b,512,512,512]
  logits = h @ W2 + b2                                  # [b,512,512,2]
  drop[b,i,j] = argmax(logits) == 1  <=>  h @ (W2[:,1]-W2[:,0]) > b2[0]-b2[1]
  attn[b,n,i,j] = (q/8 . k) - 1e9 * drop[b,i,j]

Device strategy (8 cores, SPMD), per core: batch c//4, 128 query rows.
  delta[i,j] = sum_f w2d[f] relu(a[f,i]+c[f,j]) is a 512-deep reduction per
  (i,j) pair; 33.5M relu elements must be produced elementwise (DVE/ACT) and
  streamed through the PE per core.  Both engine classes are near their
  throughput limits, so tiles are split across two legs:

  - bf16 leg (queries u < T_g of each 32-row group): DVE tensor_scalar
    (add+relu, 4x mode + per-partition scalar load, ~263ns/tile) produces
    [128f,512j] bf16 tiles; PE reduces with the shifted-Z-window trick
    (w2d at column u of a zero matrix) at ~218ns/matmul.
  - fp8 leg (queries u >= T_g): ACT Relu-with-bias (~612ns/tile) produces
    float8e4 tiles packed [128,2,512]; PE consumes them with DoubleRow
    matmuls (2 f-chunks per 216ns matmul = 2x element rate).  The fp8
    stationary holds 16*w2d, so those PSUM rows hold 16*delta.

  qk[n,i,j] is computed in fp32 right after phase A and exported raw; the
  -1e9 mask is applied on the host from the exported delta (same host
  postprocessing pass that performs the borderline fixup below).

  The drop decision is sign(delta - t).  Device tiles give delta absolute
  error ~4e-3 (bf16 leg) / ~5e-2 (fp8 leg); decision margins can be as
  small as 3e-7.  The host recomputes pairs with |delta - t| inside a
  per-leg band in float64 and patches flipped decisions exactly.
"""

import numpy as np

B, N, LQ, DK, DD = 2, 8, 512, 64, 256
F = 2 * DD          # 512 pairwise-MLP hidden dim
FC = F // 128       # 4 f-chunks
NCORES = 8
IBLK = LQ // 4      # 128 query rows per core
NEG = -1e9
TAU_BF16 = 1.2e-2   # host-recompute band, bf16-leg rows
TAU_FP8 = 9e-2      # host-recompute band, fp8-leg rows
W2D_SCALE = 16.0    # fp8 stationary scale (fp8-leg delta is 16x)
TG = (23, 22, 23, 22)   # per 32-row group: rows u < TG[g] are bf16-leg

_CACHE = {}


def _fp8_rows():
    """Per-core query rows (0..127) on the fp8 leg."""
    rows = []
    for g in range(4):
        for u in range(TG[g], 32):
            rows.append(32 * g + u)
    return np.array(rows)


def _build_nc():
    import concourse.bacc as bacc
    import concourse.tile as tile
    from concourse import mybir

    f32 = mybir.dt.float32
    bf16 = mybir.dt.bfloat16
    fp8 = mybir.dt.float8e4
    Alu = mybir.AluOpType
    Act = mybir.ActivationFunctionType
    PM = mybir.MatmulPerfMode

    nc = bacc.Bacc("TRN2", target_bir_lowering=False, debug=False,
                   num_devices=NCORES)

    # packA rows: w1b[2,512] | d0t[2,512] | w1a[2,512] | d1t[2,128]  (bf16)
    d_packA = nc.dram_tensor("packA", [128, 3328], bf16,
                             kind="ExternalInput").ap()
    d_b1c = nc.dram_tensor("b1c", [128, FC], f32, kind="ExternalInput").ap()
    d_w2cb = nc.dram_tensor("w2cb", [128, FC, 1], bf16, kind="ExternalInput").ap()
    d_w2c8 = nc.dram_tensor("w2c8", [128, 2, 2, 1], fp8, kind="ExternalInput").ap()
    d_qt = nc.dram_tensor("qt", [64, N, IBLK], f32, kind="ExternalInput").ap()
    d_kt = nc.dram_tensor("kt", [64, N, LQ], f32, kind="ExternalInput").ap()
    d_qk = nc.dram_tensor("qk", [N, IBLK, LQ], f32, kind="ExternalOutput").ap()
    d_delta = nc.dram_tensor("delta", [IBLK, LQ], f32, kind="ExternalOutput").ap()

    with tile.TileContext(nc) as tc:
        with (
            tc.tile_pool(name="const", bufs=1) as const,
            tc.tile_pool(name="tp", bufs=20) as tp,
            tc.tile_pool(name="pp", bufs=24) as pp,
            tc.tile_pool(name="op", bufs=4) as op,
            tc.tile_pool(name="gp", bufs=12) as gp,
            tc.tile_pool(name="ps", bufs=2, space="PSUM") as ps,
            tc.tile_pool(name="psq", bufs=2, space="PSUM") as psq,
        ):
            # ---- loads (all host-prearranged into SBUF layouts) ----
            sb_packA = const.tile([128, 3328], bf16)
            sb_w1b = sb_packA[:, 0:1024].rearrange("p (c f) -> p c f", c=2)
            sb_d0t = sb_packA[:, 1024:2048].rearrange("p (c f) -> p c f", c=2)
            sb_w1a = sb_packA[:, 2048:3072].rearrange("p (c f) -> p c f", c=2)
            sb_d1t = sb_packA[:, 3072:3328].rearrange("p (c f) -> p c f", c=2)
            sb_b1 = const.tile([128, FC], f32)
            sb_w2zb = const.tile([128, FC, 64], bf16)
            sb_z2 = const.tile([128, 2, 2, 64], fp8)
            sb_qt = const.tile([64, N, IBLK], f32)
            sb_kt = const.tile([64, N, LQ], f32)
            # The Z windows are mostly zeros: memset + narrow DMA of the w2d
            # column instead of shipping the zeros.
            nc.vector.memset(sb_w2zb[:], 0.0)
            nc.vector.memset(sb_z2[:], 0.0)
            # phase-A inputs split across three DMA queues (sync/SP,
            # scalar/ACT hwdge, gpsimd) -- a single queue moves only
            # ~100 GB/s, which would gate pa0 by ~5us.  q/k follow behind
            # (their consumers run much later).
            nc.sync.dma_start(out=sb_b1[:], in_=d_b1c[:])
            nc.sync.dma_start(out=sb_packA[:, 0:1024], in_=d_packA[:, 0:1024])
            nc.scalar.dma_start(out=sb_packA[:, 1024:2048],
                                in_=d_packA[:, 1024:2048])
            nc.gpsimd.dma_start(out=sb_packA[:, 2048:3328],
                                in_=d_packA[:, 2048:3328])
            nc.sync.dma_start(out=sb_w2zb[:, :, 32:33], in_=d_w2cb[:])
            nc.sync.dma_start(out=sb_z2[:, :, :, 32:33], in_=d_w2c8[:])
            nc.sync.dma_start(out=sb_kt[:], in_=d_kt[:])
            nc.gpsimd.dma_start(out=sb_qt[:], in_=d_qt[:])

            # ---- prime the ACT activation-table load (1.3us, no deps) so
            # it does not delay the first real at-chunk op.
            warm_w = const.tile([128, 32], bf16)
            warm_o = const.tile([128, 1], bf16)
            nc.vector.memset(warm_w[:], 0.0)
            nc.scalar.activation(warm_o[:], warm_w[:, 0:1], Act.Relu,
                                 bias=0.0, scale=1.0)

            # ---- phase A pieces: Ct[f,j] = (d0 @ W1b).T ;
            # At[f,i] = (d1 @ W1a).T + b1.  Emission is interleaved with the
            # start of phase C below so no engine idles (a PE gap > 3.4us
            # would re-throttle the HAM clock).
            ct = [None] * FC
            at = [None] * FC
            pa_ps = [None] * FC
            pai_ps = [None] * FC
            # all four pai chunks share one PSUM bank ([128, 4*128] f32 = 2KB)
            pai_all = ps.tile([128, FC, IBLK], f32, name="pai_all", tag="pai", bufs=1)

            def emit_pa(fc):
                pa = ps.tile([128, LQ], f32, name="pa", tag="paq", bufs=3)
                for dc in range(2):
                    nc.tensor.matmul(
                        pa[:],
                        sb_w1b[:, dc, fc * 128:(fc + 1) * 128],
                        sb_d0t[:, dc, :],
                        start=(dc == 0), stop=(dc == 1),
                    )
                pa_ps[fc] = pa
                for dc in range(2):
                    nc.tensor.matmul(
                        pai_all[:, fc, :],
                        sb_w1a[:, dc, fc * 128:(fc + 1) * 128],
                        sb_d1t[:, dc, :],
                        start=(dc == 0), stop=(dc == 1),
                        skip_group_check=True,
                    )
                pai_ps[fc] = pai_all[:, fc, :]

            def emit_ct(fc):
                ct_fc = const.tile([128, LQ], bf16, name=f"ct{fc}", tag=f"ct{fc}")
                if fc % 2 == 0:
                    nc.vector.tensor_copy(ct_fc[:], pa_ps[fc][:])
                else:
                    nc.scalar.copy(ct_fc[:], pa_ps[fc][:])
                ct[fc] = ct_fc

            def emit_at(fc):
                # 128B-aligned per-query bias columns (stride 32 floats):
                # misaligned scalar pointers cost the producers ~150ns/op
                at_fc = const.tile([128, IBLK, 32], f32, name=f"at{fc}",
                                   tag=f"at{fc}")
                nc.scalar.add(at_fc[:, :, 0], pai_ps[fc], sb_b1[:, fc:fc + 1])
                at[fc] = at_fc

            # ---- qk[n] = qT[n].T @ kT[n], exported raw (mask applied on host).
            # Bursts of 2-3 are spread across phase-C group boundaries so the
            # PSUM ring (3 bufs) never gates the PE and the copies slot into
            # the producers' budget.
            def emit_qk(n):
                pq = psq.tile([IBLK, LQ], f32, name="pq", tag="pq", bufs=2)
                nc.tensor.matmul(pq[:], sb_qt[:, n, :], sb_kt[:, n, :],
                                 start=True, stop=True, skip_group_check=True)
                qk_t = op.tile([IBLK, LQ], f32, name=f"qk{n}", tag="qk_t")
                if n % 2 == 0:
                    nc.vector.tensor_copy(qk_t[:], pq[:])
                else:
                    nc.scalar.copy(qk_t[:], pq[:])
                nc.sync.dma_start(out=d_qk[n], in_=qk_t[:])

            # ---- phase C: delta rows via shifted-window PSUM trick.
            # bf16 leg: DVE add+relu tiles, one [128,512] matmul per f-chunk.
            # fp8 leg: ACT relu tiles in [128,2,512] pairs, DoubleRow matmuls.
            # Sweep order per group interleaves the DR bursts mid-group so the
            # ACT pair ring stays shallow, and each group's delta copy is
            # emitted inside the NEXT group's stream (engine queues are strict
            # FIFO: a copy emitted at group end would stall the producer queue
            # until the PE finishes the group).
            pd_tiles = {}
            P_tiles = {}
            G_tiles = {}

            def emit_act_pair(g, pr):
                # fp8 pair tiles for (group g, chunk pair pr), emitted well
                # ahead of the matmuls that consume them so ACT never gates a
                # DR burst.
                for u in range(TG[g], 32):
                    i = 32 * g + u
                    P = pp.tile([128, 2, LQ], fp8, name="P", tag="P")
                    for s in range(2):
                        nc.scalar.activation(
                            P[:, s, :], ct[2 * pr + s][:], Act.Relu,
                            bias=at[2 * pr + s][:, i, 0:1], scale=1.0)
                    P_tiles[(g, pr, u)] = P

            def emit_delta(g):
                delta_sb = op.tile([32, LQ], f32, name="delta_sb",
                                   tag="delta_sb")
                if g % 2 == 0:
                    nc.scalar.copy(delta_sb[:], pd_tiles[g][:])
                else:
                    nc.vector.tensor_scalar(delta_sb[:], pd_tiles[g][:],
                                            0.0, None, Alu.add)
                nc.sync.dma_start(out=d_delta[32 * g:32 * g + 32, :],
                                  in_=delta_sb[:])

            for g in range(4):
                T_g = TG[g]
                pd = ps.tile([32, LQ], f32, name="pd", tag="pd")
                pd_tiles[g] = pd
                first = dict(v=True)

                def mm_start():
                    s = first["v"]
                    first["v"] = False
                    return s

                def bf_sweep(fc):
                    for u in range(T_g):
                        i = 32 * g + u
                        if (g, fc) in G_tiles and u == T_g - 1:
                            T = G_tiles[(g, fc)]
                        else:
                            T = tp.tile([128, LQ], bf16, name="T", tag="T")
                            nc.vector.tensor_scalar(
                                T[:], ct[fc][:], at[fc][:, i, 0:1], 0.0,
                                Alu.add, Alu.max)
                        nc.tensor.matmul(
                            pd[:],
                            sb_w2zb[:, fc, 32 - u:64 - u],
                            T[:],
                            start=mm_start(), stop=False,
                            skip_group_check=True,
                        )

                def dr_sweep(pr):
                    for u in range(T_g, 32):
                        nc.tensor.matmul(
                            pd[:],
                            sb_z2[:, pr, :, 32 - u:64 - u],
                            P_tiles[(g, pr, u)][:],
                            start=mm_start(),
                            stop=(pr == 1 and u == 31),
                            perf_mode=PM.DoubleRow,
                            skip_group_check=True,
                        )

                if g == 0:
                    # pipelined opening: phase-A pieces interleave with the
                    # first group so no engine idles from its first op.
                    emit_pa(0)
                    emit_pa(1)
                    emit_pa(2)
                    emit_ct(0)
                    emit_at(0)
                    emit_at(1)
                    emit_at(2)
                    bf_sweep(0)
                    emit_pa(3)
                    emit_ct(1)
                    emit_act_pair(0, 0)
                    bf_sweep(1)
                    emit_ct(2)
                    emit_at(3)
                    bf_sweep(2)
                    emit_ct(3)
                    emit_act_pair(0, 1)
                    dr_sweep(0)
                    bf_sweep(3)
                    emit_act_pair(1, 0)
                    emit_act_pair(1, 1)
                    dr_sweep(1)
                    # GPSIMD (otherwise idle) pre-produces the last bf16 tile
                    # of every later sweep, ~7.5us each, well ahead of use.
                    for gg in range(1, 4):
                        for fc in range(FC):
                            i = 32 * gg + TG[gg] - 1
                            G = gp.tile([128, LQ], bf16, name="G", tag="G")
                            nc.gpsimd.tensor_scalar(
                                G[:], ct[fc][:], at[fc][:, i, 0:1], 0.0,
                                Alu.add, Alu.max)
                            G_tiles[(gg, fc)] = G
                else:
                    bf_sweep(0)
                    bf_sweep(1)
                    emit_delta(g - 1)
                    dr_sweep(0)
                    if g < 3:
                        emit_act_pair(g + 1, 0)
                        emit_act_pair(g + 1, 1)
                    bf_sweep(2)
                    bf_sweep(3)
                    dr_sweep(1)
                for n in (range(3 * g, 3 * g + 3) if g < 2 else
                          range(6, 8) if g == 2 else []):
                    emit_qk(n)
            emit_delta(3)

    nc.compile()
    return nc


def _get_nc():
    if "nc" not in _CACHE:
        _CACHE["nc"] = _build_nc()
    return _CACHE["nc"]


def _prep_in_maps(q, k, d0, d1, W1, b1, W2, b2):
    f4 = np.float32
    import ml_dtypes

    bf = ml_dtypes.bfloat16
    f8 = ml_dtypes.float8_e4m3
    w2d = (W2[:, 1] - W2[:, 0]).astype(f4)                    # [512]
    w2cb = np.ascontiguousarray(
        w2d.reshape(FC, 128).T.astype(f4))[:, :, None].astype(bf)  # [128,4,1]
    # fp8 stationary: 16*w2d, chunk (2*pr+s) at [:, pr, s, 0]
    w2c8 = np.ascontiguousarray(
        (W2D_SCALE * w2d).reshape(2, 2, 128).transpose(2, 0, 1)
    )[:, :, :, None].astype(f8)                                    # [128,2,2,1]
    b1c = np.ascontiguousarray(b1.reshape(FC, 128).T.astype(f4))   # [128,4]
    w1a = W1[:DD].reshape(2, 128, F).transpose(1, 0, 2).astype(bf)  # [128,2,512]
    w1b = W1[DD:].reshape(2, 128, F).transpose(1, 0, 2).astype(bf)
    q8 = (q.astype(np.float64) / 8.0).astype(f4)              # exact (/8)

    in_maps = []
    for c in range(NCORES):
        b, blk = divmod(c, 4)
        isl = slice(blk * IBLK, (blk + 1) * IBLK)
        d1t = d1[b, isl, :].T.reshape(2, 128, IBLK).transpose(1, 0, 2).astype(bf)
        d0t = d0[b].T.reshape(2, 128, LQ).transpose(1, 0, 2).astype(bf)
        packA = np.ascontiguousarray(np.concatenate(
            [w1b.reshape(128, 1024), d0t.reshape(128, 1024),
             w1a.reshape(128, 1024), d1t.reshape(128, 256)], axis=1))
        qt = np.ascontiguousarray(q8[b, :, isl, :].transpose(2, 0, 1))  # [64,N,128]
        kt = np.ascontiguousarray(k[b].transpose(2, 0, 1))              # [64,N,512]
        in_maps.append({
            "packA": packA, "b1c": b1c, "w2cb": w2cb, "w2c8": w2c8,
            "qt": qt, "kt": kt,
        })
    return in_maps


def _host_finish(qk, delta, q, k, d0, d1, W1, b1, W2, b2):
    """Apply the -1e9 mask from device delta, then recompute decisions in
    float64 for pairs near the threshold and patch flipped bits exactly.

    qk:    [B, N, LQ, LQ] raw q.k/8 from device
    delta: [B, LQ, LQ] device delta; fp8-leg rows are scaled by W2D_SCALE
    """
    f8d = np.float64
    thr = float(np.float32(b2[0]) - np.float32(b2[1]))

    fp8_rows = _fp8_rows()                      # per-128-block row indices
    scale = np.ones((LQ,), dtype=np.float64)
    tau = np.full((LQ,), TAU_BF16, dtype=np.float64)
    for blk in range(4):
        scale[blk * IBLK + fp8_rows] = 1.0 / W2D_SCALE
        tau[blk * IBLK + fp8_rows] = TAU_FP8
    delta = delta.astype(np.float64) * scale[None, :, None]

    drop = delta > thr
    attn = qk + np.float32(NEG) * drop[:, None, :, :].astype(np.float32)

    d0_, d1_, W1_, b1_, W2_, b2_ = (
        x.astype(f8d) for x in (d0, d1, W1, b1, W2, b2))
    w2d = W2_[:, 1] - W2_[:, 0]
    b2diff = b2_[1] - b2_[0]

    a64 = np.einsum("bid,df->bif", d1_, W1_[:DD]) + b1_[None, None, :]
    c64 = np.einsum("bjd,df->bjf", d0_, W1_[DD:])

    border = np.argwhere(np.abs(delta - thr) < tau[None, :, None])
    nfix = 0
    for b in range(B):
        sel = border[border[:, 0] == b]
        if len(sel) == 0:
            continue
        bi, bj = sel[:, 1], sel[:, 2]
        # chunked exact recompute
        for s0 in range(0, len(bi), 8192):
            s = slice(s0, s0 + 8192)
            h = np.maximum(a64[b, bi[s]] + c64[b, bj[s]], 0.0)
            want = (h @ w2d + b2diff) > 0.0
            have = drop[b, bi[s], bj[s]]
            flip = want != have
            if not flip.any():
                continue
            fi, fj, fw = bi[s][flip], bj[s][flip], want[flip]
            nfix += len(fi)
            for ii, jj, ww in zip(fi, fj, fw):
                if ww:
                    attn[b, :, ii, jj] = qk[b, :, ii, jj] + np.float32(NEG)
                else:
                    attn[b, :, ii, jj] = qk[b, :, ii, jj]
    return attn, len(border), nfix


def kernel(q, k, d0, d1, W1, b1, W2, b2):
    from concourse import bass_utils

    q, k, d0, d1, W1, b1, W2, b2 = (
        np.asarray(x) for x in (q, k, d0, d1, W1, b1, W2, b2))
    nc = _get_nc()
    in_maps = _prep_in_maps(q, k, d0, d1, W1, b1, W2, b2)
    res = bass_utils.run_bass_kernel_spmd(nc, in_maps, list(range(NCORES)))
    outs = res.results

    qk = np.empty((B, N, LQ, LQ), dtype=np.float32)
    delta = np.empty((B, LQ, LQ), dtype=np.float32)
    for c in range(NCORES):
        b, blk = divmod(c, 4)
        isl = slice(blk * IBLK, (blk + 1) * IBLK)
        qk[b, :, isl, :] = outs[c]["qk"]
        delta[b, isl, :] = outs[c]["delta"]

    attn, _, _ = _host_finish(qk, delta, q, k, d0, d1, W1, b1, W2, b2)
    return attn


# revision 33
# speedup vs baseline: 1.6770x; 1.6770x over previous
"""Trainium2 Bass kernel for nn_DropGlobalScaledDotProductAttention.

Computation (reference semantics):
  a = d1 @ W1[:256]; c = d0 @ W1[256:]
  h[b,i,j,:] = relu(a[b,i,:] + c[b,j,:] + b1)          # [b,512,512,512]
  logits = h @ W2 + b2                                  # [b,512,512,2]
  drop[b,i,j] = argmax(logits) == 1  <=>  h @ (W2[:,1]-W2[:,0]) > b2[0]-b2[1]
  attn[b,n,i,j] = (q/8 . k) - 1e9 * drop[b,i,j]

Device strategy (8 cores, SPMD), per core: batch c//4, 128 query rows.
  delta[i,j] = sum_f w2d[f] relu(a[f,i]+c[f,j]) is a 512-deep reduction per
  (i,j) pair; 33.5M relu elements must be produced elementwise (DVE/ACT) and
  streamed through the PE per core.  Both engine classes are near their
  throughput limits, so tiles are split across two legs:

  - bf16 leg (queries u < T_g of each 32-row group): DVE tensor_scalar
    (add+relu, 4x mode + per-partition scalar load, ~263ns/tile) produces
    [128f,512j] bf16 tiles; PE reduces with the shifted-Z-window trick
    (w2d at column u of a zero matrix) at ~218ns/matmul.
  - fp8 leg (queries u >= T_g): ACT Relu-with-bias (~612ns/tile) produces
    float8e4 tiles packed [128,2,512]; PE consumes them with DoubleRow
    matmuls (2 f-chunks per 216ns matmul = 2x element rate).  The fp8
    stationary holds 16*w2d, so those PSUM rows hold 16*delta.

  qk[n,i,j] is computed in fp32 right after phase A and exported raw; the
  -1e9 mask is applied on the host from the exported delta (same host
  postprocessing pass that performs the borderline fixup below).

  The drop decision is sign(delta - t).  Device tiles give delta absolute
  error ~4e-3 (bf16 leg) / ~5e-2 (fp8 leg); decision margins can be as
  small as 3e-7.  The host recomputes pairs with |delta - t| inside a
  per-leg band in float64 and patches flipped decisions exactly.
"""

import numpy as np

B, N, LQ, DK, DD = 2, 8, 512, 64, 256
F = 2 * DD          # 512 pairwise-MLP hidden dim
FC = F // 128       # 4 f-chunks
NCORES = 8
IBLK = LQ // 4      # 128 query rows per core
NEG = -1e9
TAU_BF16 = 1.2e-2   # host-recompute band, bf16-leg rows
TAU_FP8 = 9e-2      # host-recompute band, fp8-leg rows
W2D_SCALE = 16.0    # fp8 stationary scale (fp8-leg delta is 16x)
TG = (23, 22, 23, 22)   # per 32-row group: rows u < TG[g] are bf16-leg

_CACHE = {}


def _fp8_rows():
    """Per-core query rows (0..127) on the fp8 leg."""
    rows = []
    for g in range(4):
        for u in range(TG[g], 32):
            rows.append(32 * g + u)
    return np.array(rows)


def _build_nc():
    import concourse.bacc as bacc
    import concourse.tile as tile
    from concourse import mybir

    f32 = mybir.dt.float32
    bf16 = mybir.dt.bfloat16
    fp8 = mybir.dt.float8e4
    Alu = mybir.AluOpType
    Act = mybir.ActivationFunctionType
    PM = mybir.MatmulPerfMode

    nc = bacc.Bacc("TRN2", target_bir_lowering=False, debug=False,
                   num_devices=NCORES)

    # packA rows: w1b[2,512] | d0t[2,512] | w1a[2,512] | d1t[2,128]  (bf16)
    d_packA = nc.dram_tensor("packA", [128, 3328], bf16,
                             kind="ExternalInput").ap()
    d_b1c = nc.dram_tensor("b1c", [128, FC], f32, kind="ExternalInput").ap()
    d_w2cb = nc.dram_tensor("w2cb", [128, FC, 1], bf16, kind="ExternalInput").ap()
    d_w2c8 = nc.dram_tensor("w2c8", [128, 2, 2, 1], fp8, kind="ExternalInput").ap()
    d_qt = nc.dram_tensor("qt", [64, N, IBLK], f32, kind="ExternalInput").ap()
    d_kt = nc.dram_tensor("kt", [64, N, LQ], f32, kind="ExternalInput").ap()
    d_qk = nc.dram_tensor("qk", [N, IBLK, LQ], f32, kind="ExternalOutput").ap()
    d_delta = nc.dram_tensor("delta", [IBLK, LQ], f32, kind="ExternalOutput").ap()

    with tile.TileContext(nc) as tc:
        with (
            tc.tile_pool(name="const", bufs=1) as const,
            tc.tile_pool(name="tp", bufs=20) as tp,
            tc.tile_pool(name="pp", bufs=24) as pp,
            tc.tile_pool(name="op", bufs=4) as op,
            tc.tile_pool(name="ps", bufs=2, space="PSUM") as ps,
            tc.tile_pool(name="psq", bufs=3, space="PSUM") as psq,
        ):
            # ---- loads (all host-prearranged into SBUF layouts) ----
            sb_packA = const.tile([128, 3328], bf16)
            sb_w1b = sb_packA[:, 0:1024].rearrange("p (c f) -> p c f", c=2)
            sb_d0t = sb_packA[:, 1024:2048].rearrange("p (c f) -> p c f", c=2)
            sb_w1a = sb_packA[:, 2048:3072].rearrange("p (c f) -> p c f", c=2)
            sb_d1t = sb_packA[:, 3072:3328].rearrange("p (c f) -> p c f", c=2)
            sb_b1 = const.tile([128, FC], f32)
            sb_w2zb = const.tile([128, FC, 64], bf16)
            sb_z2 = const.tile([128, 2, 2, 64], fp8)
            sb_qt = const.tile([64, N, IBLK], f32)
            sb_kt = const.tile([64, N, LQ], f32)
            # The Z windows are mostly zeros: memset + narrow DMA of the w2d
            # column instead of shipping the zeros.
            nc.vector.memset(sb_w2zb[:], 0.0)
            nc.vector.memset(sb_z2[:], 0.0)
            # phase-A inputs split across three DMA queues (sync/SP,
            # scalar/ACT hwdge, gpsimd) -- a single queue moves only
            # ~100 GB/s, which would gate pa0 by ~5us.  q/k follow behind
            # (their consumers run much later).
            nc.sync.dma_start(out=sb_b1[:], in_=d_b1c[:])
            # Load order: pa0 needs all of d0t + w1b chunk 0; pai0 needs d1t
            # + w1a chunk 0.  Critical pieces land first, split across the
            # sync and gpsimd queues; the scalar queue carries NO descriptors
            # so the at-chunk ops at its head are not delayed.
            nc.sync.dma_start(out=sb_packA[:, 1024:1536],
                              in_=d_packA[:, 1024:1536])
            nc.gpsimd.dma_start(out=sb_packA[:, 1536:2048],
                                in_=d_packA[:, 1536:2048])
            nc.gpsimd.dma_start(out=sb_packA[:, 3072:3328],
                                in_=d_packA[:, 3072:3328])
            nc.sync.dma_start(out=sb_packA[:, 0:128], in_=d_packA[:, 0:128])
            nc.sync.dma_start(out=sb_packA[:, 512:640],
                              in_=d_packA[:, 512:640])
            nc.gpsimd.dma_start(out=sb_packA[:, 2048:2176],
                                in_=d_packA[:, 2048:2176])
            nc.gpsimd.dma_start(out=sb_packA[:, 2560:2688],
                                in_=d_packA[:, 2560:2688])
            nc.sync.dma_start(out=sb_packA[:, 128:512],
                              in_=d_packA[:, 128:512])
            nc.sync.dma_start(out=sb_packA[:, 640:1024],
                              in_=d_packA[:, 640:1024])
            nc.gpsimd.dma_start(out=sb_packA[:, 2176:2560],
                                in_=d_packA[:, 2176:2560])
            nc.gpsimd.dma_start(out=sb_packA[:, 2688:3072],
                                in_=d_packA[:, 2688:3072])
            nc.sync.dma_start(out=sb_w2zb[:, :, 32:33], in_=d_w2cb[:])
            nc.sync.dma_start(out=sb_z2[:, :, :, 32:33], in_=d_w2c8[:])
            nc.sync.dma_start(out=sb_kt[:], in_=d_kt[:])
            nc.gpsimd.dma_start(out=sb_qt[:], in_=d_qt[:])

            # ---- prime the ACT activation-table load (1.3us, no deps) so
            # it does not delay the first real at-chunk op.
            warm_w = const.tile([128, 32], bf16)
            warm_o = const.tile([128, 1], bf16)
            nc.vector.memset(warm_w[:], 0.0)
            nc.scalar.activation(warm_o[:], warm_w[:, 0:1], Act.Relu,
                                 bias=0.0, scale=1.0)

            # ---- phase A pieces: Ct[f,j] = (d0 @ W1b).T ;
            # At[f,i] = (d1 @ W1a).T + b1.  Emission is interleaved with the
            # start of phase C below so no engine idles (a PE gap > 3.4us
            # would re-throttle the HAM clock).
            ct = [None] * FC
            at = [None] * FC
            pa_ps = [None] * FC
            pai_ps = [None] * FC
            # all four pai chunks share one PSUM bank ([128, 4*128] f32 = 2KB)
            pai_all = ps.tile([128, FC, IBLK], f32, name="pai_all", tag="pai", bufs=1)

            def emit_pa(fc):
                pa = ps.tile([128, LQ], f32, name="pa", tag="paq", bufs=2)
                for dc in range(2):
                    nc.tensor.matmul(
                        pa[:],
                        sb_w1b[:, dc, fc * 128:(fc + 1) * 128],
                        sb_d0t[:, dc, :],
                        start=(dc == 0), stop=(dc == 1),
                    )
                pa_ps[fc] = pa
                for dc in range(2):
                    nc.tensor.matmul(
                        pai_all[:, fc, :],
                        sb_w1a[:, dc, fc * 128:(fc + 1) * 128],
                        sb_d1t[:, dc, :],
                        start=(dc == 0), stop=(dc == 1),
                        skip_group_check=True,
                    )
                pai_ps[fc] = pai_all[:, fc, :]

            def emit_ct(fc):
                ct_fc = const.tile([128, LQ], bf16, name=f"ct{fc}", tag=f"ct{fc}")
                if fc % 2 == 0:
                    nc.vector.tensor_copy(ct_fc[:], pa_ps[fc][:])
                else:
                    nc.scalar.copy(ct_fc[:], pa_ps[fc][:])
                ct[fc] = ct_fc

            def emit_at(fc):
                # 128B-aligned per-query bias columns (stride 32 floats):
                # misaligned scalar pointers cost the producers ~150ns/op
                at_fc = const.tile([128, IBLK, 32], f32, name=f"at{fc}",
                                   tag=f"at{fc}")
                nc.scalar.add(at_fc[:, :, 0], pai_ps[fc], sb_b1[:, fc:fc + 1])
                at[fc] = at_fc

            # ---- qk[n] = qT[n].T @ kT[n], exported raw (mask applied on host).
            # Bursts of 2-3 are spread across phase-C group boundaries so the
            # PSUM ring (3 bufs) never gates the PE and the copies slot into
            # the producers' budget.
            def emit_qk(n):
                pq = psq.tile([IBLK, LQ], f32, name="pq", tag="pq", bufs=3)
                nc.tensor.matmul(pq[:], sb_qt[:, n, :], sb_kt[:, n, :],
                                 start=True, stop=True, skip_group_check=True)
                qk_t = op.tile([IBLK, LQ], f32, name=f"qk{n}", tag="qk_t")
                nc.scalar.copy(qk_t[:], pq[:])
                nc.sync.dma_start(out=d_qk[n], in_=qk_t[:])

            # ---- phase C: delta rows via shifted-window PSUM trick.
            # bf16 leg: DVE add+relu tiles, one [128,512] matmul per f-chunk.
            # fp8 leg: ACT relu tiles in [128,2,512] pairs, DoubleRow matmuls.
            # Sweep order per group interleaves the DR bursts mid-group so the
            # ACT pair ring stays shallow, and each group's delta copy is
            # emitted inside the NEXT group's stream (engine queues are strict
            # FIFO: a copy emitted at group end would stall the producer queue
            # until the PE finishes the group).
            pd_tiles = {}
            P_tiles = {}

            def emit_act_pair(g, pr):
                # fp8 pair tiles for (group g, chunk pair pr), emitted well
                # ahead of the matmuls that consume them so ACT never gates a
                # DR burst.
                for u in range(TG[g], 32):
                    i = 32 * g + u
                    P = pp.tile([128, 2, LQ], fp8, name="P", tag="P")
                    for s in range(2):
                        nc.scalar.activation(
                            P[:, s, :], ct[2 * pr + s][:], Act.Relu,
                            bias=at[2 * pr + s][:, i, 0:1], scale=1.0)
                    P_tiles[(g, pr, u)] = P

            def emit_delta(g):
                delta_sb = op.tile([32, LQ], f32, name="delta_sb",
                                   tag="delta_sb")
                nc.scalar.copy(delta_sb[:], pd_tiles[g][:])
                nc.sync.dma_start(out=d_delta[32 * g:32 * g + 32, :],
                                  in_=delta_sb[:])

            for g in range(4):
                T_g = TG[g]
                pd = ps.tile([32, LQ], f32, name="pd", tag="pd")
                pd_tiles[g] = pd
                first = dict(v=True)

                def mm_start():
                    s = first["v"]
                    first["v"] = False
                    return s

                def bf_sweep(fc, last=False):
                    for u in range(T_g):
                        i = 32 * g + u
                        T = tp.tile([128, LQ], bf16, name="T", tag="T")
                        nc.vector.tensor_scalar(
                            T[:], ct[fc][:], at[fc][:, i, 0:1], 0.0,
                            Alu.add, Alu.max)
                        nc.tensor.matmul(
                            pd[:],
                            sb_w2zb[:, fc, 32 - u:64 - u],
                            T[:],
                            start=mm_start(), stop=(last and u == T_g - 1),
                            skip_group_check=True,
                        )

                def dr_sweep(pr, last=False):
                    for u in range(T_g, 32):
                        nc.tensor.matmul(
                            pd[:],
                            sb_z2[:, pr, :, 32 - u:64 - u],
                            P_tiles[(g, pr, u)][:],
                            start=mm_start(),
                            stop=(last and u == 31),
                            perf_mode=PM.DoubleRow,
                            skip_group_check=True,
                        )

                if g == 0:
                    # pipelined opening: phase-A pieces interleave with the
                    # first group so no engine idles from its first op.
                    emit_pa(0)
                    emit_pa(1)
                    emit_pa(2)
                    emit_ct(0)
                    emit_at(0)
                    emit_at(1)
                    emit_at(2)
                    bf_sweep(0)
                    emit_pa(3)
                    emit_ct(1)
                    emit_act_pair(0, 0)
                    bf_sweep(1)
                    emit_ct(2)
                    emit_at(3)
                    bf_sweep(2)
                    emit_ct(3)
                    emit_act_pair(0, 1)
                    dr_sweep(0)
                    bf_sweep(3)
                    emit_act_pair(1, 0)
                    emit_act_pair(1, 1)
                    dr_sweep(1)
                elif g < 3:
                    bf_sweep(0)
                    bf_sweep(1)
                    emit_delta(g - 1)
                    dr_sweep(0)
                    emit_act_pair(g + 1, 0)
                    emit_act_pair(g + 1, 1)
                    bf_sweep(2)
                    bf_sweep(3)
                    dr_sweep(1, last=True)
                else:
                    bf_sweep(0)
                    bf_sweep(1)
                    emit_delta(g - 1)
                    dr_sweep(0)
                    bf_sweep(2)
                    bf_sweep(3)
                    dr_sweep(1, last=True)
                for n in (range(3 * g, 3 * g + 3) if g < 2 else
                          range(6, 8) if g == 2 else []):
                    emit_qk(n)
            emit_delta(3)

    nc.compile()
    return nc


def _get_nc():
    if "nc" not in _CACHE:
        _CACHE["nc"] = _build_nc()
    return _CACHE["nc"]


def _prep_in_maps(q, k, d0, d1, W1, b1, W2, b2):
    f4 = np.float32
    import ml_dtypes

    bf = ml_dtypes.bfloat16
    f8 = ml_dtypes.float8_e4m3
    w2d = (W2[:, 1] - W2[:, 0]).astype(f4)                    # [512]
    w2cb = np.ascontiguousarray(
        w2d.reshape(FC, 128).T.astype(f4))[:, :, None].astype(bf)  # [128,4,1]
    # fp8 stationary: 16*w2d, chunk (2*pr+s) at [:, pr, s, 0]
    w2c8 = np.ascontiguousarray(
        (W2D_SCALE * w2d).reshape(2, 2, 128).transpose(2, 0, 1)
    )[:, :, :, None].astype(f8)                                    # [128,2,2,1]
    b1c = np.ascontiguousarray(b1.reshape(FC, 128).T.astype(f4))   # [128,4]
    w1a = W1[:DD].reshape(2, 128, F).transpose(1, 0, 2).astype(bf)  # [128,2,512]
    w1b = W1[DD:].reshape(2, 128, F).transpose(1, 0, 2).astype(bf)
    q8 = (q.astype(np.float64) / 8.0).astype(f4)              # exact (/8)

    in_maps = []
    for c in range(NCORES):
        b, blk = divmod(c, 4)
        isl = slice(blk * IBLK, (blk + 1) * IBLK)
        d1t = d1[b, isl, :].T.reshape(2, 128, IBLK).transpose(1, 0, 2).astype(bf)
        d0t = d0[b].T.reshape(2, 128, LQ).transpose(1, 0, 2).astype(bf)
        packA = np.ascontiguousarray(np.concatenate(
            [w1b.reshape(128, 1024), d0t.reshape(128, 1024),
             w1a.reshape(128, 1024), d1t.reshape(128, 256)], axis=1))
        qt = np.ascontiguousarray(q8[b, :, isl, :].transpose(2, 0, 1))  # [64,N,128]
        kt = np.ascontiguousarray(k[b].transpose(2, 0, 1))              # [64,N,512]
        in_maps.append({
            "packA": packA, "b1c": b1c, "w2cb": w2cb, "w2c8": w2c8,
            "qt": qt, "kt": kt,
        })
    return in_maps


def _host_finish(qk, delta, q, k, d0, d1, W1, b1, W2, b2):
    """Apply the -1e9 mask from device delta, then recompute decisions in
    float64 for pairs near the threshold and patch flipped bits exactly.

    qk:    [B, N, LQ, LQ] raw q.k/8 from device
    delta: [B, LQ, LQ] device delta; fp8-leg rows are scaled by W2D_SCALE
    """
    f8d = np.float64
    thr = float(np.float32(b2[0]) - np.float32(b2[1]))

    fp8_rows = _fp8_rows()                      # per-128-block row indices
    scale = np.ones((LQ,), dtype=np.float64)
    tau = np.full((LQ,), TAU_BF16, dtype=np.float64)
    for blk in range(4):
        scale[blk * IBLK + fp8_rows] = 1.0 / W2D_SCALE
        tau[blk * IBLK + fp8_rows] = TAU_FP8
    delta = delta.astype(np.float64) * scale[None, :, None]

    drop = delta > thr
    attn = qk + np.float32(NEG) * drop[:, None, :, :].astype(np.float32)

    d0_, d1_, W1_, b1_, W2_, b2_ = (
        x.astype(f8d) for x in (d0, d1, W1, b1, W2, b2))
    w2d = W2_[:, 1] - W2_[:, 0]
    b2diff = b2_[1] - b2_[0]

    a64 = np.einsum("bid,df->bif", d1_, W1_[:DD]) + b1_[None, None, :]
    c64 = np.einsum("bjd,df->bjf", d0_, W1_[DD:])

    border = np.argwhere(np.abs(delta - thr) < tau[None, :, None])
    nfix = 0
    for b in range(B):
        sel = border[border[:, 0] == b]
        if len(sel) == 0:
            continue
        bi, bj = sel[:, 1], sel[:, 2]
        # chunked exact recompute
        for s0 in range(0, len(bi), 8192):
            s = slice(s0, s0 + 8192)
            h = np.maximum(a64[b, bi[s]] + c64[b, bj[s]], 0.0)
            want = (h @ w2d + b2diff) > 0.0
            have = drop[b, bi[s], bj[s]]
            flip = want != have
            if not flip.any():
                continue
            fi, fj, fw = bi[s][flip], bj[s][flip], want[flip]
            nfix += len(fi)
            for ii, jj, ww in zip(fi, fj, fw):
                if ww:
                    attn[b, :, ii, jj] = qk[b, :, ii, jj] + np.float32(NEG)
                else:
                    attn[b, :, ii, jj] = qk[b, :, ii, jj]
    return attn, len(border), nfix


def kernel(q, k, d0, d1, W1, b1, W2, b2):
    from concourse import bass_utils

    q, k, d0, d1, W1, b1, W2, b2 = (
        np.asarray(x) for x in (q, k, d0, d1, W1, b1, W2, b2))
    nc = _get_nc()
    in_maps = _prep_in_maps(q, k, d0, d1, W1, b1, W2, b2)
    res = bass_utils.run_bass_kernel_spmd(nc, in_maps, list(range(NCORES)))
    outs = res.results

    qk = np.empty((B, N, LQ, LQ), dtype=np.float32)
    delta = np.empty((B, LQ, LQ), dtype=np.float32)
    for c in range(NCORES):
        b, blk = divmod(c, 4)
        isl = slice(blk * IBLK, (blk + 1) * IBLK)
        qk[b, :, isl, :] = outs[c]["qk"]
        delta[b, isl, :] = outs[c]["delta"]

    attn, _, _ = _host_finish(qk, delta, q, k, d0, d1, W1, b1, W2, b2)
    return attn
